# revision 1
# baseline (speedup 1.0000x reference)
"""Trainium2 Bass kernel for nn_EnsembleLoss (YOLO-style ensemble loss).

Full inputs: pred (16384, 256, 12) f32, target (16384, 256, 8) f32.
Output: scalar f32 loss.

Strategy: pure data parallel over the batch dim across 8 NeuronCores
(2048 rows/core). Each core streams its 40 MiB shard through SBUF once and
computes per-partition partial sums of the three elementwise loss terms
(conf / offset / dur, folded into two accumulators) plus the quirky cls
term, which only involves the first 16384 flattened anchor rows = global
batch rows 0..31 (these live entirely in core 0's first tile). The host
sums the tiny [128, 4] per-core partials in float64.

Per-anchor math (k indexes the B*G*2 flattened anchors):
  d1 = gt_conf - pred_conf ; conf contributes (0.5 + 0.5*obj) * d1^2
  d2 = gt_off  - pred_off  ; offset contributes 5 * obj * d2^2
  d3 = sqrt(5*gt_dur) - sqrt(5*pred_dur) ; dur contributes obj * d3^2
  (obj = gt_conf, which is exactly 0.0 or 1.0)
accH accumulates 0.5*d1^2 over everything; accT accumulates
obj * (0.5*d1^2 + 5*d2^2 + d3^2).  loss_main = (accH + accT) / B.

cls term (rows 0..31, both anchors, all 256 grid cells = 16384 logit rows):
  l = obj * pred_cls (3 logits), contribution = lse(l) - l[int(obj*gt_cls)]
  summed via exp/reduce/ln on-device; host adds (sum lse - sum sel) / B.

Engine balance per 512-anchor slice: 6 DVE passes, 5 ACT passes
(squares/sqrts live on ACT — all in the `sqrt_and_others` table, so the
main loop needs no ACT table switches). DMA-bound end to end.

Roofline (verified by hw-loop For_i timing with device-resident inputs):
all 8 NCs share one trn2 chip's HBM (4 stacks x 716 GB/s = 2.86 TB/s);
streaming 320 MiB of inputs floors at ~117 us/pass, and DMA-only
variants measure 116-121 us = ~97-100% of that. The needed channels
interleave at 12-byte granularity inside every 32B HBM burst, so
channel-skipping, SWDGE bf16-cast loads, dual HWDGE rings, cls-row
redistribution, GpSimd offload, and bf16 temporaries were all measured
neutral-or-worse (build_program keeps the flags for re-testing).
Compute adds a small fixed, volume-insensitive overhead in hw-loop
timing and is otherwise fully hidden behind the DMA stream.
"""

import math

import numpy as np

import concourse.bacc as bacc
import concourse.mybir as mybir
import concourse.tile as tile
from concourse import bass_utils

F32 = mybir.dt.float32
AF = mybir.ActivationFunctionType
OP = mybir.AluOpType
AX = mybir.AxisListType

B, G, NA = 16384, 256, 2
N_CORES = 8
SHARD = B // N_CORES          # 2048 batch rows per core
PRED_W = G * NA * 6           # 3072 f32 per batch row
TGT_W = G * NA * 4            # 2048 f32 per batch row
K = G * NA                    # 512 anchors per batch row
SQ05 = math.sqrt(0.5)
SQ5 = math.sqrt(5.0)

# default build knobs (what kernel() ships)
R_DEFAULT = 1                 # batch rows per partition per tile
BUFS_DEFAULT = 4


def build_program(
    rows: int = SHARD,
    n_devices: int = N_CORES,
    reps: int = 1,
    compute: bool = True,
    R: int = R_DEFAULT,
    bufs: int = BUFS_DEFAULT,
    dual_ring: bool = False,
    dma_mode: str = "sync",   # 'sync' (HWDGE f32) | 'swdge' (gpsimd f32) | 'bf16' (gpsimd cast)
    skip_target: bool = False,
    hw_loop: bool = False,    # wrap the per-pass body in tc.For_i(0, reps, 1)
    passes_per_iter: int = 1,  # full-shard passes per For_i iteration
    gp_dur: bool = False,     # run the dur-chain subtract/mask on GpSimd
    cls_redist: bool = False,  # spread cls rows over all 128 partitions
    bf16_tmp: bool = False,   # bf16 intermediate tiles (halve engine SBUF traffic)
    skip_dur: bool = False,   # TIMING-ONLY probe: drop the dur chain
    tmp_bufs: int = 4,        # compute-intermediate pool depth
    cls_bufs: int = 1,        # cls-block pool depth (2 decouples passes)
    cls_split: bool = True,   # emit cls epilogue at pass tail, not tile 0
    cls_tail_off: int = 2,    # epilogue at t == T - cls_tail_off
):
    """One SPMD program: processes a [rows, ...] batch shard, writes
    out[128, 4] partial sums:
      col 0: sum 0.5*d1^2            (conf base, all elements)
      col 1: sum obj*inner           (conf obj + offset + dur)
      col 2: sum lse (cls rows)      (cls log-sum-exp part)
      col 3: sum sel (cls rows)      (cls selected-logit part)

    reps>1 repeats the streaming loop (overwriting the accumulators) —
    only used for loop-delta timing, not for correctness.
    """
    assert rows % (128 * R) == 0
    assert 32 % R == 0
    assert not (compute and dma_mode == "bf16"), "bf16 compute not ported yet"
    # cls redistribution: rows 0..31 re-read spread over all 128 partitions
    CJ = 4                      # row quarters -> partition p = 4*r + j
    CPM = PRED_W // CJ          # 768 pred floats per partition
    CTM = TGT_W // CJ           # 512 target floats per partition
    CQ = CPM // 6               # 128 anchors per partition
    T = rows // (128 * R)
    P = 32 // R                # partitions holding the cls rows (tile 0)
    F = R * K                  # anchors per partition per tile
    nc = bacc.Bacc(
        "TRN2", target_bir_lowering=False, debug=False, num_devices=n_devices
    )
    pred_d = nc.dram_tensor("pred", [rows, PRED_W], F32, kind="ExternalInput").ap()
    tgt_d = nc.dram_tensor("target", [rows, TGT_W], F32, kind="ExternalInput").ap()
    out_d = nc.dram_tensor("out", [128, 4], F32, kind="ExternalOutput").ap()

    tgt_dma = nc.scalar if dual_ring else nc.sync

    with tile.TileContext(nc) as tc:
        with (
            tc.tile_pool(name="pin", bufs=bufs) as pin,
            tc.tile_pool(name="tin", bufs=bufs) as tin,
            tc.tile_pool(name="tmp", bufs=tmp_bufs) as tp,
            tc.tile_pool(name="clsin", bufs=2) as cpin,
            tc.tile_pool(name="clsp", bufs=cls_bufs) as cp,
            tc.tile_pool(name="pers", bufs=1) as pp,
        ):
            accH = pp.tile([128, T], F32, tag="accH")
            accT = pp.tile([128, 3 * T], F32, tag="accT")
            out_sb = pp.tile([128, 4], F32, tag="out_sb")
            iot = pp.tile([128, 3], F32, tag="iot")

            nc.vector.memset(out_sb[:], 0.0)
            nc.vector.memset(accH[:], 0.0)
            nc.vector.memset(accT[:], 0.0)
            for c in range(3):
                nc.vector.memset(iot[:, c : c + 1], float(c))

            dma_dt = mybir.dt.bfloat16 if dma_mode == "bf16" else F32
            ld_engine = nc.gpsimd if dma_mode in ("swdge", "bf16") else nc.sync
            ld_engine2 = ld_engine if dma_mode in ("swdge", "bf16") else tgt_dma

            cls_state = {}

            def emit_cls_rest(l_t, cm, PP, QQ):
                # everything past lv/cm reads only the l_t/cm copies, so
                # it can run at the pass tail where the engines drain.
                cm_b = cm[:].unsqueeze(2).broadcast_to([PP, QQ, 3])
                iot_b = iot[0:PP, :].unsqueeze(1).broadcast_to([PP, QQ, 3])
                mq = cp.tile([PP, QQ * 3], F32, tag="mq")
                nc.vector.tensor_tensor(
                    out=mq[:].rearrange("p (q c) -> p q c", q=QQ, c=3),
                    in0=cm_b, in1=iot_b, op=OP.is_equal,
                )
                selt = cp.tile([PP, QQ * 3], F32, tag="selt")
                nc.vector.scalar_tensor_tensor(
                    out=selt[:], in0=mq[:], scalar=1.0, in1=l_t[:],
                    op0=OP.mult, op1=OP.mult,
                    accum_out=out_sb[0:PP, 3:4],
                )
                e_t = cp.tile([PP, QQ * 3], F32, tag="e_t")
                nc.scalar.activation(e_t[:], l_t[:], AF.Exp)
                se = cp.tile([PP, QQ], F32, tag="se")
                nc.vector.tensor_reduce(
                    out=se[:],
                    in_=e_t[:].rearrange("p (q c) -> p q c", q=QQ, c=3),
                    axis=AX.X, op=OP.add,
                )
                lse = cp.tile([PP, QQ], F32, tag="lse")
                nc.scalar.activation(
                    lse[:], se[:], AF.Ln, accum_out=out_sb[0:PP, 2:3]
                )

            def emit_tile(t):
                pt = pin.tile([128, R * PRED_W], dma_dt, tag="pt")
                tg = tin.tile([128, R * TGT_W], dma_dt, tag="tg")
                rows0 = t * 128 * R
                ld_engine.dma_start(
                    out=pt[:],
                    in_=pred_d[rows0 : rows0 + 128 * R, :].rearrange(
                        "(p r) m -> p (r m)", p=128, r=R
                    ),
                )
                if not skip_target:
                    ld_engine2.dma_start(
                        out=tg[:],
                        in_=tgt_d[rows0 : rows0 + 128 * R, :].rearrange(
                            "(p r) m -> p (r m)", p=128, r=R
                        ),
                    )

                if not compute:
                    return

                # anchor-major channel views: position = q*ch + c where
                # q = r*K + k runs over all F anchors with uniform stride
                pv = pt[:].rearrange("p (q c) -> p q c", q=F, c=6)
                tv = tg[:].rearrange("p (q c) -> p q c", q=F, c=4)
                po, pd, pc_ = pv[:, :, 0], pv[:, :, 1], pv[:, :, 2]
                tcf, to, td = tv[:, :, 0], tv[:, :, 2], tv[:, :, 3]

                tmp_dt = mybir.dt.bfloat16 if bf16_tmp else F32

                def mk(tag):
                    return tp.tile([128, F], tmp_dt, tag=tag, name=tag)

                d1 = mk("d1")
                hsq1 = mk("hsq1")
                d2 = mk("d2")
                sq25 = mk("sq25")
                sp = mk("sp")
                st = mk("st")
                d3 = mk("d3")
                sq35 = mk("sq35")
                tc1 = mk("tc1")
                tc2 = mk("tc2")
                tc3 = mk("tc3")

                # three independent accumulate chains (shallow critical
                # path; the scheduler overlaps them freely):
                #   conf: d1 -> 0.5*d1^2 (+accH) -> *obj (+accT)
                #   off:  d2 -> 5*d2^2            -> *obj (+accT)
                #   dur:  sqrt,sqrt -> d3 -> d3^2 -> *obj (+accT)
                nc.vector.tensor_tensor(out=d1[:], in0=tcf, in1=pc_, op=OP.subtract)
                nc.scalar.activation(
                    hsq1[:], d1[:], AF.Square, scale=SQ05,
                    accum_out=accH[:, t : t + 1],
                )
                nc.vector.scalar_tensor_tensor(
                    out=tc1[:], in0=hsq1[:], scalar=1.0, in1=tcf,
                    op0=OP.mult, op1=OP.mult,
                    accum_out=accT[:, 3 * t : 3 * t + 1],
                )
                nc.vector.tensor_tensor(out=d2[:], in0=to, in1=po, op=OP.subtract)
                nc.scalar.activation(sq25[:], d2[:], AF.Square, scale=SQ5)
                nc.vector.scalar_tensor_tensor(
                    out=tc2[:], in0=sq25[:], scalar=1.0, in1=tcf,
                    op0=OP.mult, op1=OP.mult,
                    accum_out=accT[:, 3 * t + 1 : 3 * t + 2],
                )
                if not skip_dur:
                    nc.scalar.activation(sp[:], pd, AF.Sqrt, scale=5.0)
                    nc.scalar.activation(st[:], td, AF.Sqrt, scale=5.0)
                    if gp_dur:
                        # move the two dur-chain binaries to GpSimd; fold
                        # the mask into the ACT square ((t*d3)^2 == t*d3^2
                        # since t in {0,1}).
                        nc.gpsimd.tensor_tensor(
                            out=d3[:], in0=st[:], in1=sp[:], op=OP.subtract
                        )
                        nc.gpsimd.tensor_tensor(
                            out=tc3[:], in0=d3[:], in1=tcf, op=OP.mult
                        )
                        nc.scalar.activation(
                            sq35[:], tc3[:], AF.Square,
                            accum_out=accT[:, 3 * t + 2 : 3 * t + 3],
                        )
                    else:
                        nc.vector.tensor_tensor(
                            out=d3[:], in0=st[:], in1=sp[:], op=OP.subtract
                        )
                        nc.scalar.activation(sq35[:], d3[:], AF.Square)
                        nc.vector.scalar_tensor_tensor(
                            out=tc3[:], in0=sq35[:], scalar=1.0, in1=tcf,
                            op0=OP.mult, op1=OP.mult,
                            accum_out=accT[:, 3 * t + 2 : 3 * t + 3],
                        )

                if t == 0:
                    # cls term: global batch rows 0..31 (= first 16384
                    # flattened logit rows).
                    if cls_redist:
                        # RE-READ those rows from DRAM spread over all 128
                        # partitions: partition p = 4*r + j holds cells
                        # [64j, 64j+64) of batch row r -> 128 anchors each.
                        # +640 KB DMA (+1.6%) for a 4x shorter cls chain.
                        pt2 = cpin.tile([128, CPM], F32, tag="pt2")
                        tg2 = cpin.tile([128, CTM], F32, tag="tg2")
                        nc.sync.dma_start(
                            out=pt2[:],
                            in_=pred_d[0:32, :].rearrange(
                                "r (j m) -> (r j) m", j=CJ, m=CPM
                            ),
                        )
                        tgt_dma.dma_start(
                            out=tg2[:],
                            in_=tgt_d[0:32, :].rearrange(
                                "r (j m) -> (r j) m", j=CJ, m=CTM
                            ),
                        )
                        PP, QQ = 128, CQ
                        pcl = pt2[:].rearrange("p (q c) -> p q c", q=QQ, c=6)[
                            :, :, 3:6
                        ]
                        tvc = tg2[:].rearrange("p (q c) -> p q c", q=QQ, c=4)
                    else:
                        # rows live on partitions 0..P-1 of main tile 0
                        PP, QQ = P, F
                        pcl = pt[0:P, :].rearrange("p (q c) -> p q c", q=F, c=6)[
                            :, :, 3:6
                        ]
                        tvc = tg[0:P, :].rearrange("p (q c) -> p q c", q=F, c=4)
                    obj = tvc[:, :, 0]
                    gcls = tvc[:, :, 1]

                    l_t = cp.tile([PP, QQ * 3], F32, tag="l_t")
                    cm = cp.tile([PP, QQ], F32, tag="cm")
                    nc.vector.tensor_tensor(out=cm[:], in0=obj, in1=gcls, op=OP.mult)
                    obj_b = obj.unsqueeze(2).broadcast_to([PP, QQ, 3])
                    lv = l_t[:].rearrange("p (q c) -> p q c", q=QQ, c=3)
                    nc.vector.tensor_tensor(out=lv, in0=pcl, in1=obj_b, op=OP.mult)

                    if cls_split and T > 1:
                        cls_state.update(l_t=l_t, cm=cm, PP=PP, QQ=QQ)
                    else:
                        emit_cls_rest(l_t, cm, PP, QQ)
                if cls_split and T > 1 and t == max(1, T - cls_tail_off):
                    emit_cls_rest(**cls_state)

            if hw_loop:
                with tc.For_i(0, reps, 1):
                    for _ in range(passes_per_iter):
                        for t in range(T):
                            emit_tile(t)
            else:
                for t in [t for _ in range(reps) for t in range(T)]:
                    emit_tile(t)

            # final per-partition reductions
            nc.vector.tensor_reduce(
                out=out_sb[:, 0:1], in_=accH[:], axis=AX.X, op=OP.add
            )
            nc.vector.tensor_reduce(
                out=out_sb[:, 1:2], in_=accT[:], axis=AX.X, op=OP.add
            )

            nc.sync.dma_start(out=out_d, in_=out_sb[:])

    nc.compile()
    return nc


_PROGRAM = None


def _get_program():
    global _PROGRAM
    if _PROGRAM is None:
        _PROGRAM = build_program()
    return _PROGRAM


def host_reduce(outs: list[np.ndarray]) -> np.ndarray:
    """Combine per-core [128, 4] partials into the scalar loss.
    cls partials (cols 2, 3) are only meaningful on core 0; other
    partitions/cores hold zeros there by construction on core 0, and
    other cores' cls columns are ignored entirely."""
    total = 0.0
    for o in outs:
        o64 = o.astype(np.float64)
        total += o64[:, 0].sum() + o64[:, 1].sum()
    o0 = outs[0].astype(np.float64)
    total += o0[:, 2].sum() - o0[:, 3].sum()
    return np.array(total / B, dtype=np.float32)


def kernel(pred: np.ndarray, target: np.ndarray) -> np.ndarray:
    pred = np.asarray(pred, dtype=np.float32)
    target = np.asarray(target, dtype=np.float32)
    assert pred.shape == (B, G, 12) and target.shape == (B, G, 8)
    nc = _get_program()
    in_maps = [
        {
            "pred": np.ascontiguousarray(
                pred[i * SHARD : (i + 1) * SHARD].reshape(SHARD, PRED_W),
                dtype=np.float32,
            ),
            "target": np.ascontiguousarray(
                target[i * SHARD : (i + 1) * SHARD].reshape(SHARD, TGT_W),
                dtype=np.float32,
            ),
        }
        for i in range(N_CORES)
    ]
    res = bass_utils.run_bass_kernel_spmd(nc, in_maps, core_ids=list(range(N_CORES)))
    outs = [r["out"] for r in res.results]
    return host_reduce(outs)

